# revision 2
# baseline (speedup 1.0000x reference)
"""MultiHeadAttention Trainium2 kernel, 8-core SPMD.

Sharding: core = (batch b, head-group g), b in {0,1}, g in {0..3}.
Each core computes 4 heads of one batch (tensor-parallel on heads,
data-parallel on batch). Out-projection partials are summed on host.

All matmuls run in float32r (full PE rate, ~1e-4 rel err); accumulation
is fp32 in PSUM.

Self-contained: hardcodes shapes B=2, S=2048, D=2048, H=16.
"""

import numpy as np

import concourse.bacc as bacc
import concourse.mybir as mybir
import concourse.tile as tile
from concourse.bass_utils import run_bass_kernel_spmd

B, S, D = 2, 2048, 2048
H = 16
HD = D // H          # 128 head dim
G = 4                # head groups (tensor parallel degree)
HPG = H // G         # 4 heads per group
DG = HPG * HD        # 512 features per group
NCORES = 8
NTC = D // 128       # 16 contraction chunks
NIT = S // 128       # 16 seq tiles of 128
NSC = S // 512       # 4 seq chunks of 512
SCALE = float(1.0 / np.sqrt(np.float32(S)))

F32 = mybir.dt.float32
F32R = mybir.dt.float32r
EXP = mybir.ActivationFunctionType.Exp

_CACHE = {}


def _build():
    nc = bacc.Bacc(target_bir_lowering=False, trn_type="TRN2")
    xT = nc.dram_tensor("xT", [D, S], F32R, kind="ExternalInput")
    wqT = nc.dram_tensor("wqT", [D, DG], F32R, kind="ExternalInput")
    wkT = nc.dram_tensor("wkT", [D, DG], F32R, kind="ExternalInput")
    wvT = nc.dram_tensor("wvT", [D, DG], F32R, kind="ExternalInput")
    woT = nc.dram_tensor("woT", [DG, D], F32R, kind="ExternalInput")
    bo = nc.dram_tensor("bo", [128, D], F32, kind="ExternalInput")
    mask = nc.dram_tensor("mask", [128, 128], F32R, kind="ExternalInput")
    ones = nc.dram_tensor("ones", [128, 128], F32R, kind="ExternalInput")
    y = nc.dram_tensor("y", [S, D], F32, kind="ExternalOutput")

    with tile.TileContext(nc) as tc:
        with tc.tile_pool(name="res", bufs=1) as res:
            # Resident: QT/KT per head [d=128, S], V packed 4 j-tiles per tile.
            qt = [res.tile([128, S], F32R, tag=f"qt{h}", name=f"qt{h}") for h in range(HPG)]
            kt = [res.tile([128, S], F32R, tag=f"kt{h}", name=f"kt{h}") for h in range(HPG)]
            vg = [res.tile([128, 4 * DG], F32R, tag=f"vg{j}", name=f"vg{j}") for j in range(4)]

            # ---- Phase 1a: Q/K projections (wq+wk resident, xT streamed) ----
            with tc.tile_pool(name="wqk", bufs=1) as wp:
                wq = [wp.tile([128, DG], F32R, tag=f"wq{c}", name=f"wq{c}") for c in range(NTC)]
                wk = [wp.tile([128, DG], F32R, tag=f"wk{c}", name=f"wk{c}") for c in range(NTC)]
                for c in range(NTC):
                    nc.sync.dma_start(wq[c][:], wqT[c * 128 : (c + 1) * 128, :])
                    nc.sync.dma_start(wk[c][:], wkT[c * 128 : (c + 1) * 128, :])
                with (
                    tc.tile_pool(name="xts", bufs=4) as xp,
                    tc.tile_pool(name="ps1", bufs=8, space="PSUM") as pp1,
                ):
                    for ic in range(NSC):
                        i0 = ic * 512
                        qps = [pp1.tile([128, 512], F32, tag="projps", name="projps") for _ in range(HPG)]
                        kps = [pp1.tile([128, 512], F32, tag="projps", name="projps") for _ in range(HPG)]
                        for c in range(NTC):
                            xt = xp.tile([128, 512], F32R, tag="xt", name="xt")
                            nc.sync.dma_start(
                                xt[:], xT[c * 128 : (c + 1) * 128, i0 : i0 + 512]
                            )
                            st = c == 0
                            sp = c == NTC - 1
                            for h in range(HPG):
                                nc.tensor.matmul(
                                    qps[h][:],
                                    wq[c][:, h * 128 : (h + 1) * 128],
                                    xt[:],
                                    start=st,
                                    stop=sp,
                                )
                                nc.tensor.matmul(
                                    kps[h][:],
                                    wk[c][:, h * 128 : (h + 1) * 128],
                                    xt[:],
                                    start=st,
                                    stop=sp,
                                )
                        for h in range(HPG):
                            nc.scalar.copy(qt[h][:, i0 : i0 + 512], qps[h][:])
                            nc.vector.tensor_copy(kt[h][:, i0 : i0 + 512], kps[h][:])

            # ---- Phase 1b: V projection (wv resident, xT streamed again) ----
            with tc.tile_pool(name="wv", bufs=1) as wvp:
                wv = [wvp.tile([128, DG], F32R, tag=f"wv{c}", name=f"wv{c}") for c in range(NTC)]
                for c in range(NTC):
                    nc.sync.dma_start(wv[c][:], wvT[c * 128 : (c + 1) * 128, :])
                with (
                    tc.tile_pool(name="xcol", bufs=4) as xcp,
                    tc.tile_pool(name="ps1b", bufs=4, space="PSUM") as pp2,
                ):
                    for jt in range(NIT):
                        vps = pp2.tile([128, DG], F32, tag="vps", name="vps")
                        for c in range(NTC):
                            xc = xcp.tile([128, 128], F32R, tag="xc", name="xc")
                            nc.sync.dma_start(
                                xc[:],
                                xT[c * 128 : (c + 1) * 128, jt * 128 : (jt + 1) * 128],
                            )
                            nc.tensor.matmul(
                                vps[:], xc[:], wv[c][:],
                                start=(c == 0), stop=(c == NTC - 1),
                            )
                        nc.scalar.copy(
                            vg[jt // 4][:, (jt % 4) * DG : (jt % 4 + 1) * DG], vps[:]
                        )

            # ---- Phase 2 + 3 pools ----
            with tc.tile_pool(name="ph2res", bufs=1) as p2r:
                ctxt = [p2r.tile([128, S], F32R, tag=f"ctx{h}", name=f"ctx{h}") for h in range(HPG)]
                wo = [p2r.tile([128, D], F32R, tag=f"wo{h}", name=f"wo{h}") for h in range(HPG)]
                bo_t = p2r.tile([128, D], F32, tag="bo")
                mask_t = p2r.tile([128, 128], F32R, tag="mask")
                ones_t = p2r.tile([128, 128], F32R, tag="ones")
                for h in range(HPG):
                    nc.sync.dma_start(wo[h][:], woT[h * 128 : (h + 1) * 128, :])
                nc.sync.dma_start(bo_t[:], bo[:])
                nc.sync.dma_start(mask_t[:], mask[:])
                nc.sync.dma_start(ones_t[:], ones[:])

                with (
                    tc.tile_pool(name="ph2w", bufs=3) as etp,
                    tc.tile_pool(name="ps2", bufs=2, space="PSUM") as psp,
                ):
                    # ---- Phase 2: attention (scores^T -> exp -> PV + rowsum) ----
                    for ic in range(NSC):
                        i0 = ic * 512
                        nj = 4 * (ic + 1)  # j-tiles with any j <= i in this chunk
                        for h in range(HPG):
                            ctxps = psp.tile([128, 512], F32, tag="ctxps", name="ctxps")
                            rsps = psp.tile([128, 512], F32, tag="rsps", name="rsps")
                            for jb in range(nj):
                                j0 = jb * 128
                                ist = max(i0, j0)
                                rel = ist - i0
                                stp = psp.tile([128, 512], F32, tag="stps", name="stps")
                                nc.tensor.matmul(
                                    stp[:, rel:512],
                                    kt[h][:, j0 : j0 + 128],
                                    qt[h][:, ist : i0 + 512],
                                    start=True, stop=True,
                                )
                                et = etp.tile([128, 512], F32R, tag="et", name="et")
                                nc.scalar.activation(
                                    et[:, rel:512], stp[:, rel:512], EXP,
                                    bias=0.0, scale=SCALE,
                                )
                                if j0 >= i0:
                                    nc.vector.tensor_mul(
                                        et[:, rel : rel + 128],
                                        et[:, rel : rel + 128],
                                        mask_t[:],
                                    )
                                nc.tensor.matmul(
                                    ctxps[:, rel:512],
                                    vg[jb // 4][
                                        :, (jb % 4) * DG + h * 128 : (jb % 4) * DG + (h + 1) * 128
                                    ],
                                    et[:, rel:512],
                                    start=(jb == 0), stop=(jb == nj - 1),
                                )
                                nc.tensor.matmul(
                                    rsps[:, rel:512],
                                    ones_t[:],
                                    et[:, rel:512],
                                    start=(jb == 0), stop=(jb == nj - 1),
                                )
                            rrb = etp.tile([128, 512], F32, tag="rrb", name="rrb")
                            nc.vector.reciprocal(rrb[:], rsps[:])
                            nc.vector.tensor_mul(
                                ctxt[h][:, i0 : i0 + 512], ctxps[:], rrb[:]
                            )

                    # ---- Phase 3: out-projection + bias ----
                    with (
                        tc.tile_pool(name="ysb", bufs=4) as yp,
                        tc.tile_pool(name="ps3", bufs=2, space="PSUM") as pp3,
                    ):
                        for it in range(NIT):
                            t0 = it * 128
                            for oc in range(4):
                                o0 = oc * 512
                                yps = pp3.tile([128, 512], F32, tag="yps", name="yps")
                                for h in range(HPG):
                                    nc.tensor.matmul(
                                        yps[:],
                                        ctxt[h][:, t0 : t0 + 128],
                                        wo[h][:, o0 : o0 + 512],
                                        start=(h == 0), stop=(h == HPG - 1),
                                    )
                                ysb = yp.tile([128, 512], F32, tag="ysb", name="ysb")
                                nc.vector.tensor_add(
                                    ysb[:], yps[:], bo_t[:, o0 : o0 + 512]
                                )
                                nc.sync.dma_start(
                                    y[t0 : t0 + 128, o0 : o0 + 512], ysb[:]
                                )
    nc.finalize()
    return nc


def get_nc():
    if "nc" not in _CACHE:
        _CACHE["nc"] = _build()
    return _CACHE["nc"]


def make_in_maps(inputs, w_q, w_k, w_v, w_o, b_o):
    x = np.asarray(inputs, dtype=np.float32)
    w_q = np.asarray(w_q, dtype=np.float32)
    w_k = np.asarray(w_k, dtype=np.float32)
    w_v = np.asarray(w_v, dtype=np.float32)
    w_o = np.asarray(w_o, dtype=np.float32)
    b_o = np.asarray(b_o, dtype=np.float32)

    mask = np.triu(np.ones((128, 128), dtype=np.float32))  # keep j(part) <= i(free)
    ones = np.ones((128, 128), dtype=np.float32)
    bo_rep = np.tile(b_o[None, :], (128, 1))
    bo_zero = np.zeros((128, D), dtype=np.float32)

    xTs = [np.ascontiguousarray(x[b].T) for b in range(B)]
    wqTs = [np.ascontiguousarray(w_q[g * DG : (g + 1) * DG, :].T) for g in range(G)]
    wkTs = [np.ascontiguousarray(w_k[g * DG : (g + 1) * DG, :].T) for g in range(G)]
    wvTs = [np.ascontiguousarray(w_v[g * DG : (g + 1) * DG, :].T) for g in range(G)]
    woTs = [np.ascontiguousarray(w_o[:, g * DG : (g + 1) * DG].T) for g in range(G)]

    in_maps = []
    for core in range(NCORES):
        b, g = divmod(core, G)
        in_maps.append(
            {
                "xT": xTs[b],
                "wqT": wqTs[g],
                "wkT": wkTs[g],
                "wvT": wvTs[g],
                "woT": woTs[g],
                "bo": bo_rep if g == 0 else bo_zero,
                "mask": mask,
                "ones": ones,
            }
        )
    return in_maps


def assemble(results):
    out = np.zeros((B, S, D), dtype=np.float32)
    for core in range(NCORES):
        b = core // G
        out[b] += results[core]["y"]
    return out


def kernel(inputs, w_q, w_k, w_v, w_o, b_o):
    nc = get_nc()
    in_maps = make_in_maps(inputs, w_q, w_k, w_v, w_o, b_o)
    res = run_bass_kernel_spmd(nc, in_maps, core_ids=list(range(NCORES)))
    return assemble(res.results)


# revision 4
# speedup vs baseline: 2.6602x; 2.6602x over previous
"""MultiHeadAttention Trainium2 kernel, 8-core SPMD.

Sharding: core = (batch b, head-group g), b in {0,1}, g in {0..3}.
Each core computes 4 heads of one batch (tensor-parallel on heads,
data-parallel on batch). Out-projection partials are summed on host.

All matmuls run in float32r (full PE rate, ~1e-4 rel err); accumulation
is fp32 in PSUM.

Self-contained: hardcodes shapes B=2, S=2048, D=2048, H=16.
"""

import numpy as np

import concourse.bacc as bacc
import concourse.mybir as mybir
import concourse.tile as tile
from concourse.bass_utils import run_bass_kernel_spmd

B, S, D = 2, 2048, 2048
H = 16
HD = D // H          # 128 head dim
G = 4                # head groups (tensor parallel degree)
HPG = H // G         # 4 heads per group
DG = HPG * HD        # 512 features per group
NCORES = 8
NTC = D // 128       # 16 contraction chunks
NIT = S // 128       # 16 seq tiles of 128
NSC = S // 512       # 4 seq chunks of 512
SCALE = float(1.0 / np.sqrt(np.float32(S)))

F32 = mybir.dt.float32
F32R = mybir.dt.float32r
EXP = mybir.ActivationFunctionType.Exp

_CACHE = {}


def _build(nreps=1):
    nc = bacc.Bacc(target_bir_lowering=False, trn_type="TRN2")
    xT = nc.dram_tensor("xT", [D, S], F32R, kind="ExternalInput")
    wqT = nc.dram_tensor("wqT", [D, DG], F32R, kind="ExternalInput")
    wkT = nc.dram_tensor("wkT", [D, DG], F32R, kind="ExternalInput")
    wvT = nc.dram_tensor("wvT", [D, DG], F32R, kind="ExternalInput")
    woT = nc.dram_tensor("woT", [DG, D], F32R, kind="ExternalInput")
    bo = nc.dram_tensor("bo", [128, D], F32, kind="ExternalInput")
    mask = nc.dram_tensor("mask", [128, 128], F32R, kind="ExternalInput")
    ones = nc.dram_tensor("ones", [128, 128], F32R, kind="ExternalInput")
    y = nc.dram_tensor("y", [S, D], F32, kind="ExternalOutput")

    with tile.TileContext(nc) as tc:
      for _rep in range(nreps):
        with tc.tile_pool(name="res", bufs=1) as res:
            # Resident: QT/KT per head [d=128, S], V packed 4 j-tiles per tile.
            qt = [res.tile([128, S], F32R, tag=f"qt{h}", name=f"qt{h}") for h in range(HPG)]
            kt = [res.tile([128, S], F32R, tag=f"kt{h}", name=f"kt{h}") for h in range(HPG)]
            vg = [res.tile([128, 4 * DG], F32R, tag=f"vg{j}", name=f"vg{j}") for j in range(4)]
            bo_t = res.tile([128, D], F32, tag="bo", name="bo_t")
            mask_t = res.tile([128, 128], F32R, tag="mask", name="mask_t")
            ones_t = res.tile([128, 128], F32R, tag="ones", name="ones_t")
            nc.sync.dma_start(bo_t[:], bo[:])
            nc.sync.dma_start(mask_t[:], mask[:])
            nc.sync.dma_start(ones_t[:], ones[:])

            # ---- Phase 1a: Q/K projections (wq+wk resident, xT streamed) ----
            with tc.tile_pool(name="wqk", bufs=1) as wp:
                wq = [wp.tile([128, DG], F32R, tag=f"wq{c}", name=f"wq{c}") for c in range(NTC)]
                wk = [wp.tile([128, DG], F32R, tag=f"wk{c}", name=f"wk{c}") for c in range(NTC)]
                with (
                    tc.tile_pool(name="xts", bufs=4) as xp,
                    tc.tile_pool(name="ps1", bufs=8, space="PSUM") as pp1,
                ):
                    for ic in range(NSC):
                        i0 = ic * 512
                        qps = [pp1.tile([128, 512], F32, tag="projps", name="projps") for _ in range(HPG)]
                        kps = [pp1.tile([128, 512], F32, tag="projps", name="projps") for _ in range(HPG)]
                        for c in range(NTC):
                            if ic == 0:
                                nc.sync.dma_start(
                                    wq[c][:], wqT[c * 128 : (c + 1) * 128, :]
                                )
                                nc.sync.dma_start(
                                    wk[c][:], wkT[c * 128 : (c + 1) * 128, :]
                                )
                            xt = xp.tile([128, 512], F32R, tag="xt", name="xt")
                            nc.sync.dma_start(
                                xt[:], xT[c * 128 : (c + 1) * 128, i0 : i0 + 512]
                            )
                            st = c == 0
                            sp = c == NTC - 1
                            for h in range(HPG):
                                nc.tensor.matmul(
                                    qps[h][:],
                                    wq[c][:, h * 128 : (h + 1) * 128],
                                    xt[:],
                                    start=st,
                                    stop=sp,
                                )
                                nc.tensor.matmul(
                                    kps[h][:],
                                    wk[c][:, h * 128 : (h + 1) * 128],
                                    xt[:],
                                    start=st,
                                    stop=sp,
                                )
                        for h in range(HPG):
                            nc.scalar.copy(qt[h][:, i0 : i0 + 512], qps[h][:])
                            nc.vector.tensor_copy(kt[h][:, i0 : i0 + 512], kps[h][:])

            # ---- Phase 1b: V projection (wv resident, xT streamed again) ----
            with tc.tile_pool(name="wv", bufs=1) as wvp:
                wv = [wvp.tile([128, DG], F32R, tag=f"wv{c}", name=f"wv{c}") for c in range(NTC)]
                with (
                    tc.tile_pool(name="xcol", bufs=4) as xcp,
                    tc.tile_pool(name="ps1b", bufs=4, space="PSUM") as pp2,
                ):
                    for jt in range(NIT):
                        vps = pp2.tile([128, DG], F32, tag="vps", name="vps")
                        for c in range(NTC):
                            if jt == 0:
                                nc.sync.dma_start(
                                    wv[c][:], wvT[c * 128 : (c + 1) * 128, :]
                                )
                            xc = xcp.tile([128, 128], F32R, tag="xc", name="xc")
                            nc.sync.dma_start(
                                xc[:],
                                xT[c * 128 : (c + 1) * 128, jt * 128 : (jt + 1) * 128],
                            )
                            nc.tensor.matmul(
                                vps[:], xc[:], wv[c][:],
                                start=(c == 0), stop=(c == NTC - 1),
                            )
                        nc.scalar.copy(
                            vg[jt // 4][:, (jt % 4) * DG : (jt % 4 + 1) * DG], vps[:]
                        )

            # ---- Phase 2 + 3 pools ----
            with tc.tile_pool(name="ph2res", bufs=1) as p2r:
                ctxt = [p2r.tile([128, S], F32R, tag=f"ctx{h}", name=f"ctx{h}") for h in range(HPG)]
                wo = [p2r.tile([128, D], F32R, tag=f"wo{h}", name=f"wo{h}") for h in range(HPG)]
                for h in range(HPG):
                    nc.sync.dma_start(wo[h][:], woT[h * 128 : (h + 1) * 128, :])

                with (
                    tc.tile_pool(name="ph2w", bufs=4) as etp,
                    tc.tile_pool(name="ps2", bufs=2, space="PSUM") as psp,
                ):
                    # ---- Phase 2: attention (scores^T -> exp -> PV + rowsum) ----
                    for ic in range(NSC):
                        i0 = ic * 512
                        nj = 4 * (ic + 1)  # j-tiles with any j <= i in this chunk
                        for h in range(HPG):
                            ctxps = psp.tile([128, 512], F32, tag="ctxps", name="ctxps")
                            rsps = psp.tile([128, 512], F32, tag="rsps", name="rsps")
                            for jb in range(nj):
                                j0 = jb * 128
                                ist = max(i0, j0)
                                rel = ist - i0
                                stp = psp.tile([128, 512], F32, tag="stps", name="stps")
                                nc.tensor.matmul(
                                    stp[:, rel:512],
                                    kt[h][:, j0 : j0 + 128],
                                    qt[h][:, ist : i0 + 512],
                                    start=True, stop=True,
                                )
                                et = etp.tile([128, 512], F32R, tag="et", name="et")
                                nc.scalar.activation(
                                    et[:, rel:512], stp[:, rel:512], EXP,
                                    bias=0.0, scale=SCALE,
                                )
                                if j0 >= i0:
                                    nc.vector.tensor_mul(
                                        et[:, rel : rel + 128],
                                        et[:, rel : rel + 128],
                                        mask_t[:],
                                    )
                                nc.tensor.matmul(
                                    ctxps[:, rel:512],
                                    vg[jb // 4][
                                        :, (jb % 4) * DG + h * 128 : (jb % 4) * DG + (h + 1) * 128
                                    ],
                                    et[:, rel:512],
                                    start=(jb == 0), stop=(jb == nj - 1),
                                )
                                nc.tensor.matmul(
                                    rsps[:, rel:512],
                                    ones_t[:],
                                    et[:, rel:512],
                                    start=(jb == 0), stop=(jb == nj - 1),
                                )
                            rrb = etp.tile([128, 512], F32, tag="rrb", name="rrb")
                            nc.vector.reciprocal(rrb[:], rsps[:])
                            nc.vector.tensor_mul(
                                ctxt[h][:, i0 : i0 + 512], ctxps[:], rrb[:]
                            )

                    # ---- Phase 3: out-projection + bias ----
                    with (
                        tc.tile_pool(name="ysb", bufs=4) as yp,
                        tc.tile_pool(name="ps3", bufs=2, space="PSUM") as pp3,
                    ):
                        for it in range(NIT):
                            t0 = it * 128
                            for oc in range(4):
                                o0 = oc * 512
                                yps = pp3.tile([128, 512], F32, tag="yps", name="yps")
                                for h in range(HPG):
                                    nc.tensor.matmul(
                                        yps[:],
                                        ctxt[h][:, t0 : t0 + 128],
                                        wo[h][:, o0 : o0 + 512],
                                        start=(h == 0), stop=(h == HPG - 1),
                                    )
                                ysb = yp.tile([128, 512], F32, tag="ysb", name="ysb")
                                nc.vector.tensor_add(
                                    ysb[:], yps[:], bo_t[:, o0 : o0 + 512]
                                )
                                nc.sync.dma_start(
                                    y[t0 : t0 + 128, o0 : o0 + 512], ysb[:]
                                )
    nc.finalize()
    return nc


def get_nc():
    if "nc" not in _CACHE:
        _CACHE["nc"] = _build()
    return _CACHE["nc"]


def make_in_maps(inputs, w_q, w_k, w_v, w_o, b_o):
    x = np.asarray(inputs, dtype=np.float32)
    w_q = np.asarray(w_q, dtype=np.float32)
    w_k = np.asarray(w_k, dtype=np.float32)
    w_v = np.asarray(w_v, dtype=np.float32)
    w_o = np.asarray(w_o, dtype=np.float32)
    b_o = np.asarray(b_o, dtype=np.float32)

    mask = np.triu(np.ones((128, 128), dtype=np.float32))  # keep j(part) <= i(free)
    ones = np.ones((128, 128), dtype=np.float32)
    bo_rep = np.tile(b_o[None, :], (128, 1))
    bo_zero = np.zeros((128, D), dtype=np.float32)

    xTs = [np.ascontiguousarray(x[b].T) for b in range(B)]
    wqTs = [np.ascontiguousarray(w_q[g * DG : (g + 1) * DG, :].T) for g in range(G)]
    wkTs = [np.ascontiguousarray(w_k[g * DG : (g + 1) * DG, :].T) for g in range(G)]
    wvTs = [np.ascontiguousarray(w_v[g * DG : (g + 1) * DG, :].T) for g in range(G)]
    woTs = [np.ascontiguousarray(w_o[:, g * DG : (g + 1) * DG].T) for g in range(G)]

    in_maps = []
    for core in range(NCORES):
        b, g = divmod(core, G)
        in_maps.append(
            {
                "xT": xTs[b],
                "wqT": wqTs[g],
                "wkT": wkTs[g],
                "wvT": wvTs[g],
                "woT": woTs[g],
                "bo": bo_rep if g == 0 else bo_zero,
                "mask": mask,
                "ones": ones,
            }
        )
    return in_maps


def assemble(results):
    out = np.zeros((B, S, D), dtype=np.float32)
    for core in range(NCORES):
        b = core // G
        out[b] += results[core]["y"]
    return out


def kernel(inputs, w_q, w_k, w_v, w_o, b_o):
    nc = get_nc()
    in_maps = make_in_maps(inputs, w_q, w_k, w_v, w_o, b_o)
    res = run_bass_kernel_spmd(nc, in_maps, core_ids=list(range(NCORES)))
    return assemble(res.results)


# revision 14
# speedup vs baseline: 20.7229x; 7.7900x over previous
"""MultiHeadAttention Trainium2 kernel, 8-core SPMD.

Sharding: core = (batch b, head-group g), b in {0,1}, g in {0..3}.
Each core computes 4 heads of one batch (tensor-parallel on heads,
data-parallel on batch). Out-projection partials are summed on host.

All matmuls run in float32r (full PE rate, ~1e-4 rel err); accumulation
is fp32 in PSUM.

Self-contained: hardcodes shapes B=2, S=2048, D=2048, H=16.
"""

import numpy as np

import concourse.bacc as bacc
import concourse.mybir as mybir
import concourse.tile as tile
from concourse.bass_utils import run_bass_kernel_spmd

B, S, D = 2, 2048, 2048
H = 16
HD = D // H          # 128 head dim
G = 4                # head groups (tensor parallel degree)
HPG = H // G         # 4 heads per group
DG = HPG * HD        # 512 features per group
NCORES = 8
NTC = D // 128       # 16 contraction chunks
NIT = S // 128       # 16 seq tiles of 128
NSC = S // 512       # 4 seq chunks of 512
SCALE = float(1.0 / np.sqrt(np.float32(S)))

F32 = mybir.dt.float32
F32R = mybir.dt.float32r
EXP = mybir.ActivationFunctionType.Exp

_CACHE = {}


def _build(nreps=1, trace_sim=False):
    nc = bacc.Bacc(target_bir_lowering=False, trn_type="TRN2")
    xT = nc.dram_tensor("xT", [D, S], F32R, kind="ExternalInput")
    wqT = nc.dram_tensor("wqT", [D, DG], F32R, kind="ExternalInput")
    wkT = nc.dram_tensor("wkT", [D, DG], F32R, kind="ExternalInput")
    wvT = nc.dram_tensor("wvT", [D, DG], F32R, kind="ExternalInput")
    woT = nc.dram_tensor("woT", [DG, D], F32R, kind="ExternalInput")
    bo = nc.dram_tensor("bo", [128, D], F32, kind="ExternalInput")
    mask = nc.dram_tensor("mask", [128, 128], F32R, kind="ExternalInput")
    ones = nc.dram_tensor("ones", [128, 128], F32R, kind="ExternalInput")
    y = nc.dram_tensor("y", [S, D], F32, kind="ExternalOutput")

    with tile.TileContext(nc, trace_sim=trace_sim) as tc:
      for _rep in range(nreps):
        with tc.tile_pool(name="res", bufs=1) as res:
            # Resident: QT/KT per head [d=128, S], V packed 4 j-tiles per tile.
            qt = [res.tile([128, S], F32R, tag=f"qt{h}", name=f"qt{h}") for h in range(HPG)]
            kt = [res.tile([128, S], F32R, tag=f"kt{h}", name=f"kt{h}") for h in range(HPG)]
            vg = [res.tile([128, 4 * DG], F32R, tag=f"vg{j}", name=f"vg{j}") for j in range(4)]
            bo_t = res.tile([128, D], F32, tag="bo", name="bo_t")
            mask_t = res.tile([128, 128], F32R, tag="mask", name="mask_t")
            ones_t = res.tile([128, 128], F32R, tag="ones", name="ones_t")
            nc.scalar.dma_start(bo_t[:], bo[:])
            nc.scalar.dma_start(mask_t[:], mask[:])
            nc.scalar.dma_start(ones_t[:], ones[:])

            # ---- Phase 1a: Q/K projections (wq+wk resident, xT streamed) ----
            with tc.tile_pool(name="wqk", bufs=1) as wp:
                wq = [wp.tile([128, DG], F32R, tag=f"wq{c}", name=f"wq{c}") for c in range(NTC)]
                wk = [wp.tile([128, DG], F32R, tag=f"wk{c}", name=f"wk{c}") for c in range(NTC)]
                with (
                    tc.tile_pool(name="xts", bufs=4) as xp,
                    tc.tile_pool(name="ps1", bufs=8, space="PSUM") as pp1,
                ):
                    for ic in range(NSC):
                        i0 = ic * 512
                        qps = [pp1.tile([128, 512], F32, tag="projps", name="projps") for _ in range(HPG)]
                        kps = [pp1.tile([128, 512], F32, tag="projps", name="projps") for _ in range(HPG)]
                        for c in range(NTC):
                            if ic == 0:
                                nc.sync.dma_start(
                                    wq[c][:], wqT[c * 128 : (c + 1) * 128, :]
                                )
                                nc.sync.dma_start(
                                    wk[c][:], wkT[c * 128 : (c + 1) * 128, :]
                                )
                            xt = xp.tile([128, 512], F32R, tag="xt", name="xt")
                            nc.sync.dma_start(
                                xt[:], xT[c * 128 : (c + 1) * 128, i0 : i0 + 512]
                            )
                            st = c == 0
                            sp = c == NTC - 1
                            for h in range(HPG):
                                nc.tensor.matmul(
                                    qps[h][:],
                                    wq[c][:, h * 128 : (h + 1) * 128],
                                    xt[:],
                                    start=st,
                                    stop=sp,
                                )
                                nc.tensor.matmul(
                                    kps[h][:],
                                    wk[c][:, h * 128 : (h + 1) * 128],
                                    xt[:],
                                    start=st,
                                    stop=sp,
                                )
                        for h in range(HPG):
                            nc.scalar.copy(qt[h][:, i0 : i0 + 512], qps[h][:])
                            nc.vector.tensor_copy(kt[h][:, i0 : i0 + 512], kps[h][:])

            # ---- Phase 1b: V projection (wv resident, xT streamed again) ----
            with tc.tile_pool(name="wv", bufs=1) as wvp:
                wv = [wvp.tile([128, DG], F32R, tag=f"wv{c}", name=f"wv{c}") for c in range(NTC)]
                with (
                    tc.tile_pool(name="xcol", bufs=4) as xcp,
                    tc.tile_pool(name="ps1b", bufs=4, space="PSUM") as pp2,
                ):
                    for jt in range(NIT):
                        vps = pp2.tile([128, DG], F32, tag="vps", name="vps")
                        for c in range(NTC):
                            if jt == 0:
                                nc.sync.dma_start(
                                    wv[c][:], wvT[c * 128 : (c + 1) * 128, :]
                                )
                            xc = xcp.tile([128, 128], F32R, tag="xc", name="xc")
                            nc.sync.dma_start(
                                xc[:],
                                xT[c * 128 : (c + 1) * 128, jt * 128 : (jt + 1) * 128],
                            )
                            nc.tensor.matmul(
                                vps[:], xc[:], wv[c][:],
                                start=(c == 0), stop=(c == NTC - 1),
                            )
                        nc.scalar.copy(
                            vg[jt // 4][:, (jt % 4) * DG : (jt % 4 + 1) * DG], vps[:]
                        )

            # ---- Phase 2 + 3 pools ----
            with tc.tile_pool(name="ph2res", bufs=1) as p2r:
                ctxt = [p2r.tile([128, S], F32R, tag=f"ctx{h}", name=f"ctx{h}") for h in range(HPG)]
                wo = [p2r.tile([128, D], F32R, tag=f"wo{h}", name=f"wo{h}") for h in range(HPG)]
                for h in range(HPG):
                    nc.sync.dma_start(wo[h][:], woT[h * 128 : (h + 1) * 128, :])

                with (
                    tc.tile_pool(name="ph2w", bufs=4) as etp,
                    tc.tile_pool(name="ps2", bufs=2, space="PSUM") as psp,
                ):
                    # ---- Phase 2: attention (scores^T -> exp -> PV + rowsum) ----
                    for ic in range(NSC):
                        i0 = ic * 512
                        nj = 4 * (ic + 1)  # j-tiles with any j <= i in this chunk
                        for h in range(HPG):
                            ctxps = psp.tile([128, 512], F32, tag="ctxps", name="ctxps", bufs=2)
                            rsps = psp.tile([128, 512], F32, tag="rsps", name="rsps", bufs=2)
                            for jb in range(nj):
                                j0 = jb * 128
                                ist = max(i0, j0)
                                rel = ist - i0
                                stp = psp.tile([128, 512], F32, tag="stps", name="stps", bufs=4)
                                nc.tensor.matmul(
                                    stp[:, rel:512],
                                    kt[h][:, j0 : j0 + 128],
                                    qt[h][:, ist : i0 + 512],
                                    start=True, stop=True,
                                )
                                et = etp.tile([128, 512], F32R, tag="et", name="et")
                                nc.scalar.activation(
                                    et[:, rel:512], stp[:, rel:512], EXP,
                                    bias=0.0, scale=SCALE,
                                )
                                if j0 >= i0:
                                    nc.vector.tensor_mul(
                                        et[:, rel : rel + 128],
                                        et[:, rel : rel + 128],
                                        mask_t[:],
                                    )
                                nc.tensor.matmul(
                                    ctxps[:, rel:512],
                                    vg[jb // 4][
                                        :, (jb % 4) * DG + h * 128 : (jb % 4) * DG + (h + 1) * 128
                                    ],
                                    et[:, rel:512],
                                    start=(jb == 0), stop=(jb == nj - 1),
                                )
                                nc.tensor.matmul(
                                    rsps[:, rel:512],
                                    ones_t[:],
                                    et[:, rel:512],
                                    start=(jb == 0), stop=(jb == nj - 1),
                                )
                            rrb = etp.tile([128, 512], F32, tag="rrb", name="rrb")
                            nc.vector.reciprocal(rrb[:], rsps[:])
                            nc.vector.tensor_mul(
                                ctxt[h][:, i0 : i0 + 512], ctxps[:], rrb[:]
                            )

                    # ---- Phase 3: out-projection + bias ----
                    with (
                        tc.tile_pool(name="ysb", bufs=4) as yp,
                        tc.tile_pool(name="ps3", bufs=2, space="PSUM") as pp3,
                    ):
                        for it in range(NIT):
                            t0 = it * 128
                            for oc in range(4):
                                o0 = oc * 512
                                yps = pp3.tile([128, 512], F32, tag="yps", name="yps")
                                for h in range(HPG):
                                    nc.tensor.matmul(
                                        yps[:],
                                        ctxt[h][:, t0 : t0 + 128],
                                        wo[h][:, o0 : o0 + 512],
                                        start=(h == 0), stop=(h == HPG - 1),
                                    )
                                ysb = yp.tile([128, 512], F32, tag="ysb", name="ysb")
                                nc.vector.tensor_add(
                                    ysb[:], yps[:], bo_t[:, o0 : o0 + 512]
                                )
                                nc.sync.dma_start(
                                    y[t0 : t0 + 128, o0 : o0 + 512], ysb[:]
                                )
    nc.finalize()
    return nc


def get_nc():
    if "nc" not in _CACHE:
        _CACHE["nc"] = _build()
    return _CACHE["nc"]


def make_in_maps(inputs, w_q, w_k, w_v, w_o, b_o):
    x = np.asarray(inputs, dtype=np.float32)
    w_q = np.asarray(w_q, dtype=np.float32)
    w_k = np.asarray(w_k, dtype=np.float32)
    w_v = np.asarray(w_v, dtype=np.float32)
    w_o = np.asarray(w_o, dtype=np.float32)
    b_o = np.asarray(b_o, dtype=np.float32)

    mask = np.triu(np.ones((128, 128), dtype=np.float32))  # keep j(part) <= i(free)
    ones = np.ones((128, 128), dtype=np.float32)
    bo_rep = np.tile(b_o[None, :], (128, 1))
    bo_zero = np.zeros((128, D), dtype=np.float32)

    xTs = [np.ascontiguousarray(x[b].T) for b in range(B)]
    wqTs = [np.ascontiguousarray(w_q[g * DG : (g + 1) * DG, :].T) for g in range(G)]
    wkTs = [np.ascontiguousarray(w_k[g * DG : (g + 1) * DG, :].T) for g in range(G)]
    wvTs = [np.ascontiguousarray(w_v[g * DG : (g + 1) * DG, :].T) for g in range(G)]
    woTs = [np.ascontiguousarray(w_o[:, g * DG : (g + 1) * DG].T) for g in range(G)]

    in_maps = []
    for core in range(NCORES):
        b, g = divmod(core, G)
        in_maps.append(
            {
                "xT": xTs[b],
                "wqT": wqTs[g],
                "wkT": wkTs[g],
                "wvT": wvTs[g],
                "woT": woTs[g],
                "bo": bo_rep if g == 0 else bo_zero,
                "mask": mask,
                "ones": ones,
            }
        )
    return in_maps


def assemble(results):
    out = np.zeros((B, S, D), dtype=np.float32)
    for core in range(NCORES):
        b = core // G
        out[b] += results[core]["y"]
    return out


def kernel(inputs, w_q, w_k, w_v, w_o, b_o):
    nc = get_nc()
    in_maps = make_in_maps(inputs, w_q, w_k, w_v, w_o, b_o)
    res = run_bass_kernel_spmd(nc, in_maps, core_ids=list(range(NCORES)))
    return assemble(res.results)
